# revision 61
# baseline (speedup 1.0000x reference)
"""Quaternion multi-head attention (nn_Attention_53395033424361) on 8 TRN2 NeuronCores.

Sharding: core = b*2 + hg  (b in 0..3 batches, hg in 0..1 head-groups of 4 heads).
Each core computes, for its batch b and its 4 heads, the attention output and a
partial output-projection y_part[b] (contraction over its heads' 384 features).
Host unshard: y[b] = y_part[core 2b] + y_part[core 2b+1] + bias.

All quaternion (Hamilton) structure is folded into host-assembled effective
weight matrices.  Key optimizations vs the original version:
  - all matmul operands bf16 (halved DMA, FWL weight loads, 2x/4x DVE modes;
    note TRN2 matmul throughput itself stays 1 col/cycle -- fp32-PSUM drain)
  - proj-B computes only the plain V_r for all 4 heads in one N=384 pass;
    the 4 quaternion V-variants are free-dim chunk moves on the DVE (the
    original spent 4x the proj-B matmul FLOPs materializing them)
  - deep cross-engine software pipeline: AV matmuls trail scores/exp by 3 kt
    steps (carried across blocks and heads); attention-output PSUM tiles
    spill to SBUF on the DVE immediately so PSUM banks recycle without
    waiting on the normalization chain; the softmax 1/r broadcast reads its
    r-row straight from the spilled SBUF copy (single bf16 rank-1 matmul)
  - head h+1's q/K projections and variant moves drain as interleaved tasks
    inside head h's attention (gated off first blocks so in-flight weight
    DMAs cannot head-of-line-block the PE queue)
  - ramp: x/weights split across the two HW-DGE rings (sync + scalar) and
    the token-half-1 projections drain as ordered early-tasks inside head
    0's first block; tail: proj-C for tokens 0-511 runs inside head 3's
    attention, and tokens 512-1023 precompute their heads-0..2 partials
    early, leaving only the head-3 matmul + fused add at the drain
"""

import contextlib
import ctypes
import os
import sys
import types

import ml_dtypes
import numpy as np

import concourse.bass as bass
import concourse.mybir as mybir
import concourse.tile as tile
from concourse import bacc, bass_utils

B, N, DIM, H = 4, 1024, 768, 8
HD = DIM // H          # 96 head dim
QC = HD // 4           # 24 quaternion sub-chunk
NCORES = 8
HPC = H // 2           # heads per core (4)
DT = 6                 # 768 / 128 contraction tiles
F32 = mybir.dt.float32
BF16 = mybir.dt.bfloat16

_PROGRAM_CACHE = {}


# ----------------------------------------------------------------------------
# Host-side weight assembly
# ----------------------------------------------------------------------------

def _build_w_eff(wr, wi, wj, wk):
    row_r = np.concatenate([wr, wi, wj, wk], axis=1)
    row_i = np.concatenate([-wi, wr, -wk, wj], axis=1)
    row_j = np.concatenate([-wj, wk, wr, -wi], axis=1)
    row_k = np.concatenate([-wk, -wj, wi, wr], axis=1)
    return np.concatenate([row_r, row_i, row_j, row_k], axis=0)


def _k_variants(Wk):
    c = [Wk[:, i*QC:(i+1)*QC] for i in range(4)]
    return [
        np.concatenate([c[0], -c[1], -c[2], -c[3]], 1),
        np.concatenate([c[1], c[0], c[3], -c[2]], 1),
        np.concatenate([c[2], -c[3], c[0], c[1]], 1),
        np.concatenate([c[3], c[2], -c[1], c[0]], 1),
    ]


def _v_variants(Wv):
    c = [Wv[:, i*QC:(i+1)*QC] for i in range(4)]
    return [
        np.concatenate([c[0], c[1], c[2], c[3]], 1),
        np.concatenate([-c[1], c[0], -c[3], c[2]], 1),
        np.concatenate([-c[2], c[3], c[0], -c[1]], 1),
        np.concatenate([-c[3], -c[2], c[1], c[0]], 1),
    ]


def _host_prepare(inputs):
    """Returns (in_maps, bp) -- one input dict per core."""
    f32 = np.float32
    bf = ml_dtypes.bfloat16
    x = np.ascontiguousarray(np.asarray(inputs["x"], f32))
    W = _build_w_eff(*[np.asarray(inputs[f"wqkv_{c}"], f32) for c in "rijk"])
    Wp = _build_w_eff(*[np.asarray(inputs[f"wp_{c}"], f32) for c in "rijk"])
    bp = np.asarray(inputs["bp"], f32)

    def pad32(w):
        # [768, 96] -> [768, 128]: each 24-col chunk lands at a 32-col slot
        # (zero-filled) so on-device partition slices stay 32-aligned
        out = np.zeros((w.shape[0], 128), f32)
        for e in range(4):
            out[:, 32*e:32*e+QC] = w[:, QC*e:QC*(e+1)]
        return out

    # Per-head device weights:
    #  wa [768, 256]: [K_r(pad32) | q*scale(pad32)]; K_i/j/k built on device
    #  wv (per core) [768, 384]: plain V_r for the core's 4 heads; the V
    #  quaternion variants are cheap free-dim chunk moves on device
    wa_heads, wv_heads = [], []
    for h in range(H):
        Wq = W[:, h*HD:(h+1)*HD] * f32(HD ** -0.5)
        Wk = W[:, DIM + h*HD: DIM + (h+1)*HD]
        Wv = W[:, 2*DIM + h*HD: 2*DIM + (h+1)*HD]
        wa_heads.append(np.concatenate(
            [pad32(_k_variants(Wk)[0]), pad32(Wq)], axis=1))
        wv_heads.append(Wv)

    def ptile(w):
        # [768, F] -> partition-major [128, 6*F] so the device DMA is contiguous
        f = w.shape[1]
        return np.ascontiguousarray(
            w.reshape(DT, 128, f).transpose(1, 0, 2).reshape(128, DT * f))

    def ptile_th(w):
        # [768, 1024] -> [128, 2, 6, 512] (token-half outermost) so each half
        # is one fully-contiguous 6 KiB/partition DMA
        return np.ascontiguousarray(
            w.reshape(DT, 128, 2, 512).transpose(1, 2, 0, 3).reshape(128, -1))

    in_maps = []
    for core in range(NCORES):
        b, hg = core // 2, core % 2
        hs = hg * HPC
        wp_c = Wp[hs*HD:(hs+HPC)*HD, :]                                # [384, 768]
        in_maps.append({
            "xt": ptile_th(x[b].T).astype(bf),                         # [128, 6144]
            "wa": np.ascontiguousarray(np.concatenate(
                [ptile(wa_heads[hs+i]) for i in range(HPC)], axis=1)).astype(bf),
            "wv": ptile(np.concatenate(
                [wv_heads[hs+i] for i in range(HPC)], axis=1)).astype(bf),
            "wp": np.ascontiguousarray(
                wp_c.reshape(HPC, HD, DIM).transpose(1, 0, 2)
                .reshape(HD, HPC * DIM)).astype(bf),                   # [96, 3072]
        })
    return in_maps, bp


# ----------------------------------------------------------------------------
# Device program (SPMD -- identical on all 8 cores)
# ----------------------------------------------------------------------------

def _build_program():
    nc = bacc.Bacc("TRN2", target_bir_lowering=False, debug=False,
                   num_devices=NCORES)
    xt_d = nc.dram_tensor("xt", [128, DT * N], BF16, kind="ExternalInput").ap()
    wa_d = nc.dram_tensor("wa", [128, HPC * DT * 256], BF16, kind="ExternalInput").ap()
    wv_d = nc.dram_tensor("wv", [128, DT * HPC * HD], BF16, kind="ExternalInput").ap()
    wp_d = nc.dram_tensor("wp", [HD, HPC * DIM], BF16, kind="ExternalInput").ap()
    y_d = nc.dram_tensor("y", [N, DIM], BF16, kind="ExternalOutput").ap()

    EXP = mybir.ActivationFunctionType.Exp

    with tile.TileContext(nc) as tc:
        with (
            tc.tile_pool(name="const", bufs=1) as cpool,
            tc.tile_pool(name="wstream", bufs=2) as wpool,
            tc.tile_pool(name="kvar", bufs=2) as kvar_pool,
            tc.tile_pool(name="u", bufs=6) as u_pool,
            tc.tile_pool(name="small", bufs=2) as spool,
            tc.tile_pool(name="ysb", bufs=2) as y_pool,
            tc.tile_pool(name="ps_big", bufs=2, space="PSUM") as ps_big,
            tc.tile_pool(name="ps_o", bufs=2, space="PSUM") as ps_o,
            tc.tile_pool(name="ps_proj", bufs=2, space="PSUM") as ps_proj,
        ):
            # --- persistent tiles -------------------------------------------------
            # first head's weights land before x so its proj can start early
            wa0_sb = wpool.tile([128, DT, 256], BF16, tag="wa", name="wa_0")
            nc.scalar.dma_start(
                wa0_sb[:],
                wa_d[:, 0:DT*256].rearrange("p (o f) -> p o f", o=DT))
            wv_sb = cpool.tile([128, DT, HPC * HD], BF16)
            # xt_sb [128, token-half, d, 512]: each half is one contiguous DMA
            xt_sb = cpool.tile([128, 2, DT, 512], BF16)

            def xt_dma(th):
                if th == 0:
                    for dh in range(2):
                        nc.sync.dma_start(
                            xt_sb[:, 0, dh*3:dh*3+3, :],
                            xt_d.rearrange("p (t o f) -> p t o f", t=2, o=DT)
                            [:, 0, dh*3:dh*3+3])
                else:
                    nc.sync.dma_start(
                        xt_sb[:, th, :, :],
                        xt_d.rearrange("p (t o f) -> p t o f", t=2, o=DT)[:, th])
            nc.scalar.dma_start(
                wv_sb[:], wv_d.rearrange("p (o f) -> p o f", o=DT))
            xt_dma(0)
            xt_dma(1)

            wp_sb = cpool.tile([128, HPC, DIM], BF16)
            nc.gpsimd.memset(wp_sb[HD:128, :, :], 0.0)
            nc.scalar.dma_start(wp_sb[0:HD, :, :],
                              wp_d.rearrange("p (h g) -> p h g", h=HPC))

            # sel: rank-1 selector (ones row at partition 96) for the 1/r
            # partition-broadcast matmul; rhs is the spilled po_sb r-row, which
            # also lives at partition 96 (lhsT/rhs base partitions must match)
            sel = cpool.tile([128, HD], BF16)
            nc.gpsimd.memset(sel[:], 0.0)
            nc.gpsimd.memset(sel[96:97, :], 1.0)

            # o^T accumulator for all 4 heads [96 feat, head, tokens]
            o_sb = cpool.tile([128, HPC, N], BF16)
            nc.gpsimd.memset(o_sb[HD:128, :, :], 0.0)

            # v_all [keys, head, comp, key-tile, 98]: all heads' V (keys-major).
            # comp 0 = plain V_r from proj-B; comps 1-3 are signed free-dim
            # chunk moves of comp 0. col 96 = ones (softmax sum via AV matmul).
            v_all = cpool.tile([128, HPC, 4, 8, 98], BF16)
            nc.gpsimd.memset(v_all[:, :, :, :, HD:HD+1], 1.0)

            # V quaternion variants: comp c chunk e <- (source chunk, sign)
            V_VAR_TABLE = [
                [(1, -1.0), (0, 1.0), (3, -1.0), (2, 1.0)],   # V_i
                [(2, -1.0), (3, 1.0), (0, 1.0), (1, -1.0)],   # V_j
                [(3, -1.0), (2, -1.0), (1, 1.0), (0, 1.0)],   # V_k
            ]

            def v_var_moves(h, v):
                for e, (g, sign) in enumerate(V_VAR_TABLE[v]):
                    nc.vector.tensor_scalar_mul(
                        v_all[:, h, 1 + v, :, e*QC:(e+1)*QC],
                        v_all[:, h, 0, :, g*QC:(g+1)*QC],
                        sign)

            def v_var_moves_half(h, v, half):
                # key-tile-half variant moves (ramp path: head 0's second half
                # of x is still in flight when its first AVs are emitted)
                sl = slice(half*4, half*4+4)
                for e, (g, sign) in enumerate(V_VAR_TABLE[v]):
                    nc.vector.tensor_scalar_mul(
                        v_all[:, h, 1 + v, sl, e*QC:(e+1)*QC],
                        v_all[:, h, 0, sl, g*QC:(g+1)*QC],
                        sign)

            def proj_b(tt):
                # all 4 heads' V_r for one key tile in a single matmul pass
                psB = ps_proj.tile([128, 512], F32, tag="psp",
                                   name=f"psB_{tt}")
                for d in range(DT):
                    nc.tensor.matmul(
                        psB[:, 0:HPC*HD],
                        lhsT=xt_sb[:, tt//4, d, (tt % 4)*128:(tt % 4 + 1)*128],
                        rhs=wv_sb[:, d, :],
                        start=(d == 0), stop=(d == DT - 1))
                nc.vector.tensor_copy(
                    v_all[:, :, 0, tt, 0:HD],
                    psB[:, 0:HPC*HD].rearrange("p (h f) -> p h f", h=HPC))

            # ---------------- background (interleavable) proj units --------------
            kvars = {}

            def stream_weights(h):
                wa_sb = wpool.tile([128, DT, 256], BF16, tag="wa",
                                   name=f"wa_{h}")
                nc.sync.dma_start(
                    wa_sb[:],
                    wa_d[:, h*DT*256:(h+1)*DT*256]
                    .rearrange("p (o f) -> p o f", o=DT))
                return wa_sb

            # K_i/j/k from K_r: signed 32-row chunk moves (DVE, bf16).
            K_VAR_TABLE = [
                [(1, -1.0), (0, 1.0), (3, -1.0), (2, 1.0)],   # K_i
                [(2, -1.0), (3, 1.0), (0, 1.0), (1, -1.0)],   # K_j
                [(3, -1.0), (2, -1.0), (1, 1.0), (0, 1.0)],   # K_k
            ]

            def proj_tasks(h, wa_sb, split_var=False, include_v=True):
                """Closures for head h's q/K projections + V variant moves,
                emitted one at a time inside head h-1's attention loop."""
                tasks = []

                # kvar_sb [128, 5, 1024]: slot 0 = K_r^T, 1-3 = K_i/j/k^T,
                # 4 = q^T (scaled)
                kvar_sb = kvar_pool.tile([128, 5, N], BF16, tag="kvar",
                                         name=f"kvar_{h}")
                kvars[h] = kvar_sb

                def proj_a(blk, th, h=h, wa_sb=wa_sb, kvar_sb=kvar_sb):
                    dst_blk = 0 if blk == 0 else 4
                    psA = ps_proj.tile([128, 512], F32, tag="psp",
                                       name=f"psA_{h}_{blk}_{th}")
                    for d in range(DT):
                        nc.tensor.matmul(
                            psA[:, :],
                            lhsT=wa_sb[:, d, blk*128:(blk+1)*128],
                            rhs=xt_sb[:, th, d, :],
                            start=(d == 0), stop=(d == DT - 1))
                    nc.vector.tensor_copy(
                        kvar_sb[:, dst_blk, th*512:(th+1)*512], psA[:, :])

                def k_var_moves(v, hh, kvar_sb=kvar_sb):
                    # hh: key half (split so the first scores can start before
                    # the second half of x has arrived)
                    sl = slice(hh*512, (hh+1)*512)
                    for t, (s, sign) in enumerate(K_VAR_TABLE[v]):
                        nc.vector.tensor_scalar_mul(
                            kvar_sb[32*t:32*t+32, 1 + v, sl],
                            kvar_sb[32*s:32*s+32, 0, sl],
                            sign)

                if split_var:
                    for th in range(2):
                        tasks.append(lambda th=th: proj_a(0, th))
                        tasks.append(lambda th=th: proj_a(1, th))
                        for v in range(3):
                            tasks.append(lambda v=v, th=th: k_var_moves(v, th))
                else:
                    for blk in range(2):
                        for th in range(2):
                            tasks.append(lambda blk=blk, th=th:
                                         proj_a(blk, th))
                    for v in range(3):
                        tasks.append(lambda v=v: (k_var_moves(v, 0),
                                                  k_var_moves(v, 1)))
                if include_v:
                    for v in range(3):
                        tasks.append(lambda v=v, h=h: v_var_moves(h, v))
                return tasks

            # Ramp: only the token/key-half-0 path runs up front (x half 1 is
            # still in flight); the half-1 projections drain as early_tasks
            # (two per kt) inside head 0's first attention block, ordered so
            # every score/AV consumer is emitted after its producer.
            tasks0 = proj_tasks(0, wa0_sb, split_var=True, include_v=False)
            for t in tasks0[:5]:     # psA(blk,th0) x2 + kvm(v,th0) x3
                t()
            for tt in range(4):
                proj_b(tt)
            for v in range(3):
                v_var_moves_half(0, v, 0)
            pa01, pa11, kvm01, kvm11, kvm21 = tasks0[5:]
            early_tasks = [
                lambda: (pa01(), proj_b(4)),
                lambda: (pa11(), proj_b(5)),
                lambda: (kvm01(), proj_b(6)),
                lambda: (proj_b(7), v_var_moves_half(0, 0, 1)),
                lambda: (v_var_moves_half(0, 1, 1), v_var_moves_half(0, 2, 1)),
                lambda: kvm11(),
                lambda: kvm21(),
            ]

            pending_tasks = []
            triggers = {}
            # AV matmuls run two kt steps behind scores/exp (software pipeline
            # carried across blocks and heads) so neither PE nor ACT serializes
            # at block boundaries; spill/norm of a block fire early in the next
            pending_av = []
            pending_norm = [None]
            pending_spill = [None]

            def attention(h):
                kvar_sb = kvars[h]
                oaccs = {}
                for th in range(2):
                    tok = slice(th*512, (th+1)*512)
                    oacc = spool.tile([128, 512], BF16, tag="oacc",
                                      name=f"oacc_{h}_{th}")
                    oaccs[th] = oacc
                    for cp in range(2):
                        po = [ps_o.tile([128, 512], F32, tag="pso",
                                        name=f"po_{h}_{th}_{cp}_{ci}")
                              for ci in range(2)]
                        # bf16 SBUF spill target (r-row at partition 96 incl.)
                        po_sb = [spool.tile([128, 512], BF16, tag=f"posb{ci}",
                                            name=f"posb_{h}_{th}_{cp}_{ci}")
                                 for ci in range(2)]
                        for kt in range(8):
                            psS = ps_big.tile([128, 1024], F32, tag="psb",
                                              name=f"psS_{h}_{th}_{cp}_{kt}")
                            for ci in range(2):
                                nc.tensor.matmul(
                                    psS[:, ci*512:(ci+1)*512],
                                    lhsT=kvar_sb[:, 2*cp+ci,
                                                 kt*128:(kt+1)*128],
                                    rhs=kvar_sb[:, 4, tok],
                                    start=True, stop=True)
                            u = u_pool.tile([128, 1024], BF16, tag="u",
                                            name=f"u_{h}_{th}_{cp}_{kt}")
                            nc.scalar.activation(u[:], psS[:], EXP)
                            if len(pending_av) == 4:
                                pending_av.pop(0)()
                            if kt == 3 and pending_spill[0] is not None:
                                # fires after the previous block's last AV
                                # (drained from pending_av just above)
                                pending_spill[0]()
                                pending_spill[0] = None
                            if kt == 4 and pending_norm[0] is not None:
                                # deferred so the po spill has finished before
                                # the PE hits the psR broadcast matmuls
                                pending_norm[0]()
                                pending_norm[0] = None

                            def av(h=h, cp=cp, kt=kt, po=po, u=u):
                                for ci in range(2):
                                    nc.tensor.matmul(
                                        po[ci][0:HD+1, :],
                                        lhsT=v_all[:, h, 2*cp+ci, kt, 0:HD+1],
                                        rhs=u[:, ci*512:(ci+1)*512],
                                        start=(kt == 0), stop=(kt == 7))
                            pending_av.append(av)
                            # fill the exp-wait bubble with proj work; head 0's
                            # half-1 units (early_tasks) may run in the first
                            # block, the next head's units may not (their
                            # weights may still be in flight and a waiting
                            # matmul would head-of-line-block the PE queue)
                            if early_tasks:
                                early_tasks.pop(0)()
                            elif pending_tasks and (th, cp) != (0, 0):
                                pending_tasks.pop(0)()
                            extra = triggers.pop((h, th, cp, kt), None)
                            if extra:
                                pending_tasks.extend(extra)

                        def spill(po=po, po_sb=po_sb):
                            # po -> SBUF (DVE; the scalar engine stays
                            # dedicated to the exp stream)
                            for ci in range(2):
                                nc.vector.tensor_copy(po_sb[ci][0:HD+1, :],
                                                      po[ci][0:HD+1, :])

                        def norm(th=th, cp=cp, po_sb=po_sb, tok=tok, h=h):
                            # softmax normalization: o += po[c][:96] * (1/r_c)
                            oacc = oaccs[th]
                            psR = ps_big.tile([128, 1024], F32, tag="psb",
                                              name=f"psR_{h}_{th}_{cp}")
                            for ci in range(2):
                                nc.tensor.matmul(
                                    psR[0:HD, ci*512:(ci+1)*512],
                                    lhsT=sel[96:97, :],
                                    rhs=po_sb[ci][HD:HD+1, :],
                                    start=True, stop=True,
                                    tile_position=(96, 0))
                            rbc = spool.tile([128, 1024], F32, tag="rbc",
                                             name=f"rbc_{h}_{th}_{cp}")
                            nc.vector.reciprocal_approx_fast(
                                rbc[0:HD, :], psR[0:HD, :])
                            for ci in range(2):
                                idx = 2*cp + ci
                                if idx == 0:
                                    nc.vector.tensor_mul(
                                        oacc[0:HD, :],
                                        po_sb[ci][0:HD, :],
                                        rbc[0:HD, ci*512:(ci+1)*512])
                                else:
                                    tmp = spool.tile(
                                        [128, 512], BF16, tag="otmp",
                                        name=f"otmp_{h}_{th}_{cp}_{ci}")
                                    nc.vector.tensor_mul(
                                        tmp[0:HD, :], po_sb[ci][0:HD, :],
                                        rbc[0:HD, ci*512:(ci+1)*512])
                                    dst = (o_sb[0:HD, h, tok] if idx == 3
                                           else oacc[0:HD, :])
                                    nc.vector.tensor_add(
                                        dst, oacc[0:HD, :], tmp[0:HD, :])

                        pending_spill[0] = spill
                        pending_norm[0] = norm
                # last head: drain the AV/spill/norm pipeline before proj-C
                if h == HPC - 1:
                    while pending_av:
                        pending_av.pop(0)()
                    if pending_spill[0] is not None:
                        pending_spill[0]()
                        pending_spill[0] = None
                    if pending_norm[0] is not None:
                        pending_norm[0]()
                        pending_norm[0] = None

            def proj_c(tt):
                y_sb = y_pool.tile([128, DIM], BF16, tag="ysb",
                                   name=f"ysb_{tt}")
                for gh in range(2):
                    psY = ps_proj.tile([128, 512], F32, tag="psp",
                                       name=f"psY_{tt}_{gh}")
                    for hh in range(HPC):
                        nc.tensor.matmul(
                            psY[:, 0:384],
                            lhsT=o_sb[:, hh, tt*128:(tt+1)*128],
                            rhs=wp_sb[:, hh, gh*384:(gh+1)*384],
                            start=(hh == 0), stop=(hh == HPC - 1))
                    nc.vector.tensor_copy(y_sb[:, gh*384:(gh+1)*384],
                                          psY[:, 0:384])
                nc.sync.dma_start(y_d[tt*128:(tt+1)*128, :], y_sb[:])

            # proj-C for tokens 512-1023 is gated on the very last norm, so
            # the heads-0..2 part of its contraction runs early (during head
            # 3's attention) and only the head-3 matmul remains in the tail
            y012_sb = cpool.tile([128, 4, DIM], BF16)

            def proj_c_early(tt):
                for gh in range(2):
                    psY = ps_proj.tile([128, 512], F32, tag="psp",
                                       name=f"psY0_{tt}_{gh}")
                    for hh in range(HPC - 1):
                        nc.tensor.matmul(
                            psY[:, 0:384],
                            lhsT=o_sb[:, hh, tt*128:(tt+1)*128],
                            rhs=wp_sb[:, hh, gh*384:(gh+1)*384],
                            start=(hh == 0), stop=(hh == HPC - 2))
                    nc.vector.tensor_copy(y012_sb[:, tt-4, gh*384:(gh+1)*384],
                                          psY[:, 0:384])

            def proj_c_rest(tt):
                y_sb = y_pool.tile([128, DIM], BF16, tag="ysb",
                                   name=f"ysb_{tt}")
                for gh in range(2):
                    psY = ps_proj.tile([128, 512], F32, tag="psp",
                                       name=f"psY3_{tt}_{gh}")
                    nc.tensor.matmul(
                        psY[:, 0:384],
                        lhsT=o_sb[:, HPC-1, tt*128:(tt+1)*128],
                        rhs=wp_sb[:, HPC-1, gh*384:(gh+1)*384],
                        start=True, stop=True)
                    nc.vector.scalar_tensor_tensor(
                        y_sb[:, gh*384:(gh+1)*384],
                        psY[:, 0:384], 1.0,
                        y012_sb[:, tt-4, gh*384:(gh+1)*384],
                        op0=mybir.AluOpType.mult, op1=mybir.AluOpType.add)
                nc.sync.dma_start(y_d[tt*128:(tt+1)*128, :], y_sb[:])

            # tokens 0-511 (tt 0-3) are fully normalized once head 3's th=0
            # norms fire (early in its th=1 loop) -> run those proj-C tiles
            # inside head 3's attention instead of in the tail; the heads-0..2
            # partials for tt 4-7 (ready once head 2's th=1 norm fired, i.e.
            # during head 3's first block) run there too
            triggers[(HPC-1, 0, 1, 2)] = [lambda tt=tt: proj_c_early(tt)
                                          for tt in range(4, 8)]
            triggers[(HPC-1, 1, 0, 3)] = [lambda tt=tt: proj_c(tt)
                                          for tt in range(4)]

            for h in range(HPC):
                if h + 1 < HPC:
                    wa_sb = stream_weights(h + 1)
                    pending_tasks.extend(proj_tasks(h + 1, wa_sb))
                attention(h)
                # drain any leftover tasks before next head's attention
                while pending_tasks:
                    pending_tasks.pop(0)()

            # --- proj-C: remaining token tiles (head-3 part only) ---------------
            for tt in range(4, 8):
                proj_c_rest(tt)

    nc.compile()
    return nc


def _get_program():
    if "nc" not in _PROGRAM_CACHE:
        _PROGRAM_CACHE["nc"] = _build_program()
    return _PROGRAM_CACHE["nc"]


# ----------------------------------------------------------------------------
# NTFF profiling hook (axon containers without antenv.axon_hooks)
# ----------------------------------------------------------------------------

def _install_ntff_hook():
    """Provide antenv.axon_hooks backed by libaxon_pjrt.so so that
    run_bass_kernel_spmd(trace=True) can capture NTFF profiles under axon.
    Returns True if tracing is possible."""
    try:
        from antenv.axon_hooks import get_axon_ntff_profile_hook  # noqa: F401
        return True
    except ImportError:
        pass
    so_path = "/opt/axon/libaxon_pjrt.so"
    if not os.path.exists(so_path):
        return False
    lib = ctypes.CDLL(so_path)
    if not hasattr(lib, "axon_start_nrt_profile"):
        return False
    lib.axon_start_nrt_profile.argtypes = [
        ctypes.POINTER(ctypes.c_int64), ctypes.c_size_t]
    lib.axon_start_nrt_profile.restype = ctypes.c_int64
    lib.axon_stop_nrt_profile.argtypes = [ctypes.c_char_p]
    lib.axon_stop_nrt_profile.restype = ctypes.c_int64

    @contextlib.contextmanager
    def _hook(output_dir, device_ids):
        import jax
        jax.devices()
        if device_ids:
            ids = (ctypes.c_int64 * len(device_ids))(*device_ids)
            rc = lib.axon_start_nrt_profile(ids, len(device_ids))
        else:
            rc = lib.axon_start_nrt_profile(None, 0)
        if rc != 0:
            raise RuntimeError(f"axon_start_nrt_profile rc={rc}")
        try:
            yield
        finally:
            n = lib.axon_stop_nrt_profile(str(output_dir).encode())
            print(f"profile: {n} file(s) written to {output_dir}",
                  file=sys.stderr)

    mod = types.ModuleType("antenv.axon_hooks")
    _state = {"hook": _hook}
    mod.set_axon_ntff_profile_hook = lambda h: _state.__setitem__("hook", h)
    mod.get_axon_ntff_profile_hook = lambda: _state["hook"]
    sys.modules["antenv.axon_hooks"] = mod
    import antenv
    antenv.axon_hooks = mod
    return True


# ----------------------------------------------------------------------------
# Entry point
# ----------------------------------------------------------------------------

def kernel(trace=False, **inputs):
    nc = _get_program()
    in_maps, bp = _host_prepare(inputs)
    if trace:
        trace = _install_ntff_hook()
    res = bass_utils.run_bass_kernel_spmd(
        nc, in_maps, core_ids=list(range(NCORES)), trace=trace)
    y = np.empty((B, N, DIM), np.float32)
    for b in range(B):
        y[b] = (res.results[2*b]["y"].astype(np.float32)
                + res.results[2*b+1]["y"].astype(np.float32) + bp)
    if trace:
        kernel.last_results = res
    return y


# revision 62
# speedup vs baseline: 1.0088x; 1.0088x over previous
"""Quaternion multi-head attention (nn_Attention_53395033424361) on 8 TRN2 NeuronCores.

Sharding: core = b*2 + hg  (b in 0..3 batches, hg in 0..1 head-groups of 4 heads).
Each core computes, for its batch b and its 4 heads, the attention output and a
partial output-projection y_part[b] (contraction over its heads' 384 features).
Host unshard: y[b] = y_part[core 2b] + y_part[core 2b+1] + bias.

All quaternion (Hamilton) structure is folded into host-assembled effective
weight matrices.  Key optimizations vs the original version:
  - all matmul operands bf16 (halved DMA, FWL weight loads, 2x/4x DVE modes;
    note TRN2 matmul throughput itself stays 1 col/cycle -- fp32-PSUM drain)
  - proj-B computes only the plain V_r for all 4 heads in one N=384 pass;
    the 4 quaternion V-variants are free-dim chunk moves on the DVE (the
    original spent 4x the proj-B matmul FLOPs materializing them)
  - deep cross-engine software pipeline: AV matmuls trail scores/exp by 3 kt
    steps (carried across blocks and heads); attention-output PSUM tiles
    spill to SBUF on the DVE immediately so PSUM banks recycle without
    waiting on the normalization chain; the softmax 1/r broadcast reads its
    r-row straight from the spilled SBUF copy (single bf16 rank-1 matmul)
  - head h+1's q/K projections and variant moves drain as interleaved tasks
    inside head h's attention (gated off first blocks so in-flight weight
    DMAs cannot head-of-line-block the PE queue)
  - ramp: x/weights split across the two HW-DGE rings (sync + scalar) and
    the token-half-1 projections drain as ordered early-tasks inside head
    0's first block; tail: proj-C for tokens 0-511 runs inside head 3's
    attention, and tokens 512-1023 precompute their heads-0..2 partials
    early, leaving only the head-3 matmul + fused add at the drain
"""

import contextlib
import ctypes
import os
import sys
import types

import ml_dtypes
import numpy as np

import concourse.bass as bass
import concourse.mybir as mybir
import concourse.tile as tile
from concourse import bacc, bass_utils

B, N, DIM, H = 4, 1024, 768, 8
HD = DIM // H          # 96 head dim
QC = HD // 4           # 24 quaternion sub-chunk
NCORES = 8
HPC = H // 2           # heads per core (4)
DT = 6                 # 768 / 128 contraction tiles
F32 = mybir.dt.float32
BF16 = mybir.dt.bfloat16

_PROGRAM_CACHE = {}


# ----------------------------------------------------------------------------
# Host-side weight assembly
# ----------------------------------------------------------------------------

def _build_w_eff(wr, wi, wj, wk):
    row_r = np.concatenate([wr, wi, wj, wk], axis=1)
    row_i = np.concatenate([-wi, wr, -wk, wj], axis=1)
    row_j = np.concatenate([-wj, wk, wr, -wi], axis=1)
    row_k = np.concatenate([-wk, -wj, wi, wr], axis=1)
    return np.concatenate([row_r, row_i, row_j, row_k], axis=0)


def _k_variants(Wk):
    c = [Wk[:, i*QC:(i+1)*QC] for i in range(4)]
    return [
        np.concatenate([c[0], -c[1], -c[2], -c[3]], 1),
        np.concatenate([c[1], c[0], c[3], -c[2]], 1),
        np.concatenate([c[2], -c[3], c[0], c[1]], 1),
        np.concatenate([c[3], c[2], -c[1], c[0]], 1),
    ]


def _v_variants(Wv):
    c = [Wv[:, i*QC:(i+1)*QC] for i in range(4)]
    return [
        np.concatenate([c[0], c[1], c[2], c[3]], 1),
        np.concatenate([-c[1], c[0], -c[3], c[2]], 1),
        np.concatenate([-c[2], c[3], c[0], -c[1]], 1),
        np.concatenate([-c[3], -c[2], c[1], c[0]], 1),
    ]


def _host_prepare(inputs):
    """Returns (in_maps, bp) -- one input dict per core."""
    f32 = np.float32
    bf = ml_dtypes.bfloat16
    x = np.ascontiguousarray(np.asarray(inputs["x"], f32))
    W = _build_w_eff(*[np.asarray(inputs[f"wqkv_{c}"], f32) for c in "rijk"])
    Wp = _build_w_eff(*[np.asarray(inputs[f"wp_{c}"], f32) for c in "rijk"])
    bp = np.asarray(inputs["bp"], f32)

    def pad32(w):
        # [768, 96] -> [768, 128]: each 24-col chunk lands at a 32-col slot
        # (zero-filled) so on-device partition slices stay 32-aligned
        out = np.zeros((w.shape[0], 128), f32)
        for e in range(4):
            out[:, 32*e:32*e+QC] = w[:, QC*e:QC*(e+1)]
        return out

    # Per-head device weights:
    #  wa [768, 256]: [K_r(pad32) | q*scale(pad32)]; K_i/j/k built on device
    #  wv (per core) [768, 384]: plain V_r for the core's 4 heads; the V
    #  quaternion variants are cheap free-dim chunk moves on device
    wa_heads, wv_heads = [], []
    for h in range(H):
        Wq = W[:, h*HD:(h+1)*HD] * f32(HD ** -0.5)
        Wk = W[:, DIM + h*HD: DIM + (h+1)*HD]
        Wv = W[:, 2*DIM + h*HD: 2*DIM + (h+1)*HD]
        wa_heads.append(np.concatenate(
            [pad32(_k_variants(Wk)[0]), pad32(Wq)], axis=1))
        wv_heads.append(Wv)

    def ptile(w):
        # [768, F] -> partition-major [128, 6*F] so the device DMA is contiguous
        f = w.shape[1]
        return np.ascontiguousarray(
            w.reshape(DT, 128, f).transpose(1, 0, 2).reshape(128, DT * f))

    def ptile_th(w):
        # [768, 1024] -> [128, 2, 6, 512] (token-half outermost) so each half
        # is one fully-contiguous 6 KiB/partition DMA
        return np.ascontiguousarray(
            w.reshape(DT, 128, 2, 512).transpose(1, 2, 0, 3).reshape(128, -1))

    in_maps = []
    for core in range(NCORES):
        b, hg = core // 2, core % 2
        hs = hg * HPC
        wp_c = Wp[hs*HD:(hs+HPC)*HD, :]                                # [384, 768]
        in_maps.append({
            "xt": ptile_th(x[b].T).astype(bf),                         # [128, 6144]
            "wa": np.ascontiguousarray(np.concatenate(
                [ptile(wa_heads[hs+i]) for i in range(HPC)], axis=1)).astype(bf),
            "wv": ptile(np.concatenate(
                [wv_heads[hs+i] for i in range(HPC)], axis=1)).astype(bf),
            "wp": np.ascontiguousarray(
                wp_c.reshape(HPC, HD, DIM).transpose(1, 0, 2)
                .reshape(HD, HPC * DIM)).astype(bf),                   # [96, 3072]
        })
    return in_maps, bp


# ----------------------------------------------------------------------------
# Device program (SPMD -- identical on all 8 cores)
# ----------------------------------------------------------------------------

def _build_program():
    nc = bacc.Bacc("TRN2", target_bir_lowering=False, debug=False,
                   num_devices=NCORES)
    xt_d = nc.dram_tensor("xt", [128, DT * N], BF16, kind="ExternalInput").ap()
    wa_d = nc.dram_tensor("wa", [128, HPC * DT * 256], BF16, kind="ExternalInput").ap()
    wv_d = nc.dram_tensor("wv", [128, DT * HPC * HD], BF16, kind="ExternalInput").ap()
    wp_d = nc.dram_tensor("wp", [HD, HPC * DIM], BF16, kind="ExternalInput").ap()
    y_d = nc.dram_tensor("y", [N, DIM], BF16, kind="ExternalOutput").ap()

    EXP = mybir.ActivationFunctionType.Exp

    with tile.TileContext(nc) as tc:
        with (
            tc.tile_pool(name="const", bufs=1) as cpool,
            tc.tile_pool(name="wstream", bufs=2) as wpool,
            tc.tile_pool(name="kvar", bufs=2) as kvar_pool,
            tc.tile_pool(name="u", bufs=5) as u_pool,
            tc.tile_pool(name="small", bufs=2) as spool,
            tc.tile_pool(name="ysb", bufs=2) as y_pool,
            tc.tile_pool(name="ps_big", bufs=2, space="PSUM") as ps_big,
            tc.tile_pool(name="ps_o", bufs=2, space="PSUM") as ps_o,
            tc.tile_pool(name="ps_proj", bufs=2, space="PSUM") as ps_proj,
        ):
            # --- persistent tiles -------------------------------------------------
            # first head's weights land before x so its proj can start early
            wa0_sb = wpool.tile([128, DT, 256], BF16, tag="wa", name="wa_0")
            nc.scalar.dma_start(
                wa0_sb[:],
                wa_d[:, 0:DT*256].rearrange("p (o f) -> p o f", o=DT))
            wv_sb = cpool.tile([128, DT, HPC * HD], BF16)
            # xt_sb [128, token-half, d, 512]: each half is one contiguous DMA
            xt_sb = cpool.tile([128, 2, DT, 512], BF16)

            def xt_dma(th):
                if th == 0:
                    for dh in range(2):
                        nc.sync.dma_start(
                            xt_sb[:, 0, dh*3:dh*3+3, :],
                            xt_d.rearrange("p (t o f) -> p t o f", t=2, o=DT)
                            [:, 0, dh*3:dh*3+3])
                else:
                    nc.sync.dma_start(
                        xt_sb[:, th, :, :],
                        xt_d.rearrange("p (t o f) -> p t o f", t=2, o=DT)[:, th])
            nc.scalar.dma_start(
                wv_sb[:], wv_d.rearrange("p (o f) -> p o f", o=DT))
            xt_dma(0)
            xt_dma(1)

            wp_sb = cpool.tile([128, HPC, DIM], BF16)
            nc.gpsimd.memset(wp_sb[HD:128, :, :], 0.0)
            nc.scalar.dma_start(wp_sb[0:HD, :, :],
                              wp_d.rearrange("p (h g) -> p h g", h=HPC))

            # sel: rank-1 selector (ones row at partition 96) for the 1/r
            # partition-broadcast matmul; rhs is the spilled po_sb r-row, which
            # also lives at partition 96 (lhsT/rhs base partitions must match)
            sel = cpool.tile([128, HD], BF16)
            nc.gpsimd.memset(sel[:], 0.0)
            nc.gpsimd.memset(sel[96:97, :], 1.0)

            # o^T accumulator for all 4 heads [96 feat, head, tokens]
            o_sb = cpool.tile([128, HPC, N], BF16)
            nc.gpsimd.memset(o_sb[HD:128, :, :], 0.0)

            # v_all [keys, head, comp, key-tile, 98]: all heads' V (keys-major).
            # comp 0 = plain V_r from proj-B; comps 1-3 are signed free-dim
            # chunk moves of comp 0. col 96 = ones (softmax sum via AV matmul).
            v_all = cpool.tile([128, HPC, 4, 8, 98], BF16)
            nc.gpsimd.memset(v_all[:, :, :, :, HD:HD+1], 1.0)

            # V quaternion variants: comp c chunk e <- (source chunk, sign)
            V_VAR_TABLE = [
                [(1, -1.0), (0, 1.0), (3, -1.0), (2, 1.0)],   # V_i
                [(2, -1.0), (3, 1.0), (0, 1.0), (1, -1.0)],   # V_j
                [(3, -1.0), (2, -1.0), (1, 1.0), (0, 1.0)],   # V_k
            ]

            def v_var_moves(h, v):
                for e, (g, sign) in enumerate(V_VAR_TABLE[v]):
                    nc.vector.tensor_scalar_mul(
                        v_all[:, h, 1 + v, :, e*QC:(e+1)*QC],
                        v_all[:, h, 0, :, g*QC:(g+1)*QC],
                        sign)

            def v_var_moves_half(h, v, half):
                # key-tile-half variant moves (ramp path: head 0's second half
                # of x is still in flight when its first AVs are emitted)
                sl = slice(half*4, half*4+4)
                for e, (g, sign) in enumerate(V_VAR_TABLE[v]):
                    nc.vector.tensor_scalar_mul(
                        v_all[:, h, 1 + v, sl, e*QC:(e+1)*QC],
                        v_all[:, h, 0, sl, g*QC:(g+1)*QC],
                        sign)

            def proj_b(tt):
                # all 4 heads' V_r for one key tile in a single matmul pass
                psB = ps_proj.tile([128, 512], F32, tag="psp",
                                   name=f"psB_{tt}")
                for d in range(DT):
                    nc.tensor.matmul(
                        psB[:, 0:HPC*HD],
                        lhsT=xt_sb[:, tt//4, d, (tt % 4)*128:(tt % 4 + 1)*128],
                        rhs=wv_sb[:, d, :],
                        start=(d == 0), stop=(d == DT - 1))
                nc.vector.tensor_copy(
                    v_all[:, :, 0, tt, 0:HD],
                    psB[:, 0:HPC*HD].rearrange("p (h f) -> p h f", h=HPC))

            # ---------------- background (interleavable) proj units --------------
            kvars = {}

            def stream_weights(h):
                wa_sb = wpool.tile([128, DT, 256], BF16, tag="wa",
                                   name=f"wa_{h}")
                nc.sync.dma_start(
                    wa_sb[:],
                    wa_d[:, h*DT*256:(h+1)*DT*256]
                    .rearrange("p (o f) -> p o f", o=DT))
                return wa_sb

            # K_i/j/k from K_r: signed 32-row chunk moves (DVE, bf16).
            K_VAR_TABLE = [
                [(1, -1.0), (0, 1.0), (3, -1.0), (2, 1.0)],   # K_i
                [(2, -1.0), (3, 1.0), (0, 1.0), (1, -1.0)],   # K_j
                [(3, -1.0), (2, -1.0), (1, 1.0), (0, 1.0)],   # K_k
            ]

            def proj_tasks(h, wa_sb, split_var=False, include_v=True):
                """Closures for head h's q/K projections + V variant moves,
                emitted one at a time inside head h-1's attention loop."""
                tasks = []

                # kvar_sb [128, 5, 1024]: slot 0 = K_r^T, 1-3 = K_i/j/k^T,
                # 4 = q^T (scaled)
                kvar_sb = kvar_pool.tile([128, 5, N], BF16, tag="kvar",
                                         name=f"kvar_{h}")
                kvars[h] = kvar_sb

                def proj_a(blk, th, h=h, wa_sb=wa_sb, kvar_sb=kvar_sb):
                    dst_blk = 0 if blk == 0 else 4
                    psA = ps_proj.tile([128, 512], F32, tag="psp",
                                       name=f"psA_{h}_{blk}_{th}")
                    for d in range(DT):
                        nc.tensor.matmul(
                            psA[:, :],
                            lhsT=wa_sb[:, d, blk*128:(blk+1)*128],
                            rhs=xt_sb[:, th, d, :],
                            start=(d == 0), stop=(d == DT - 1))
                    nc.vector.tensor_copy(
                        kvar_sb[:, dst_blk, th*512:(th+1)*512], psA[:, :])

                def k_var_moves(v, hh, kvar_sb=kvar_sb):
                    # hh: key half (split so the first scores can start before
                    # the second half of x has arrived)
                    sl = slice(hh*512, (hh+1)*512)
                    for t, (s, sign) in enumerate(K_VAR_TABLE[v]):
                        nc.vector.tensor_scalar_mul(
                            kvar_sb[32*t:32*t+32, 1 + v, sl],
                            kvar_sb[32*s:32*s+32, 0, sl],
                            sign)

                if split_var:
                    for th in range(2):
                        tasks.append(lambda th=th: proj_a(0, th))
                        tasks.append(lambda th=th: proj_a(1, th))
                        for v in range(3):
                            tasks.append(lambda v=v, th=th: k_var_moves(v, th))
                else:
                    for blk in range(2):
                        for th in range(2):
                            tasks.append(lambda blk=blk, th=th:
                                         proj_a(blk, th))
                    for v in range(3):
                        tasks.append(lambda v=v: (k_var_moves(v, 0),
                                                  k_var_moves(v, 1)))
                if include_v:
                    for v in range(3):
                        tasks.append(lambda v=v, h=h: v_var_moves(h, v))
                return tasks

            # Ramp: only the token/key-half-0 path runs up front (x half 1 is
            # still in flight); the half-1 projections drain as early_tasks
            # (two per kt) inside head 0's first attention block, ordered so
            # every score/AV consumer is emitted after its producer.
            tasks0 = proj_tasks(0, wa0_sb, split_var=True, include_v=False)
            for t in tasks0[:5]:     # psA(blk,th0) x2 + kvm(v,th0) x3
                t()
            for tt in range(4):
                proj_b(tt)
            for v in range(3):
                v_var_moves_half(0, v, 0)
            pa01, pa11, kvm01, kvm11, kvm21 = tasks0[5:]
            early_tasks = [
                lambda: (pa01(), proj_b(4)),
                lambda: (pa11(), proj_b(5)),
                lambda: (kvm01(), proj_b(6)),
                lambda: (proj_b(7), v_var_moves_half(0, 0, 1)),
                lambda: (v_var_moves_half(0, 1, 1), v_var_moves_half(0, 2, 1)),
                lambda: kvm11(),
                lambda: kvm21(),
            ]

            pending_tasks = []
            triggers = {}
            # AV matmuls run two kt steps behind scores/exp (software pipeline
            # carried across blocks and heads) so neither PE nor ACT serializes
            # at block boundaries; spill/norm of a block fire early in the next
            pending_av = []
            pending_norm = [None]
            pending_spill = [None]

            def attention(h):
                kvar_sb = kvars[h]
                oaccs = {}
                for th in range(2):
                    tok = slice(th*512, (th+1)*512)
                    oacc = spool.tile([128, 512], BF16, tag="oacc",
                                      name=f"oacc_{h}_{th}")
                    oaccs[th] = oacc
                    for cp in range(2):
                        po = [ps_o.tile([128, 512], F32, tag="pso",
                                        name=f"po_{h}_{th}_{cp}_{ci}")
                              for ci in range(2)]
                        # bf16 SBUF spill target (r-row at partition 96 incl.)
                        po_sb = [spool.tile([128, 512], BF16, tag=f"posb{ci}",
                                            name=f"posb_{h}_{th}_{cp}_{ci}")
                                 for ci in range(2)]
                        for kt in range(8):
                            psS = ps_big.tile([128, 1024], F32, tag="psb",
                                              name=f"psS_{h}_{th}_{cp}_{kt}")
                            for ci in range(2):
                                nc.tensor.matmul(
                                    psS[:, ci*512:(ci+1)*512],
                                    lhsT=kvar_sb[:, 2*cp+ci,
                                                 kt*128:(kt+1)*128],
                                    rhs=kvar_sb[:, 4, tok],
                                    start=True, stop=True)
                            u = u_pool.tile([128, 1024], BF16, tag="u",
                                            name=f"u_{h}_{th}_{cp}_{kt}")
                            nc.scalar.activation(u[:], psS[:], EXP)
                            if len(pending_av) == 3:
                                pending_av.pop(0)()
                            if kt == 2 and pending_spill[0] is not None:
                                # fires after the previous block's last AV
                                # (drained from pending_av just above)
                                pending_spill[0]()
                                pending_spill[0] = None
                            if kt == 3 and pending_norm[0] is not None:
                                # deferred so the po spill has finished before
                                # the PE hits the psR broadcast matmuls
                                pending_norm[0]()
                                pending_norm[0] = None

                            def av(h=h, cp=cp, kt=kt, po=po, u=u):
                                for ci in range(2):
                                    nc.tensor.matmul(
                                        po[ci][0:HD+1, :],
                                        lhsT=v_all[:, h, 2*cp+ci, kt, 0:HD+1],
                                        rhs=u[:, ci*512:(ci+1)*512],
                                        start=(kt == 0), stop=(kt == 7))
                            pending_av.append(av)
                            # fill the exp-wait bubble with proj work; head 0's
                            # half-1 units (early_tasks) may run in the first
                            # block, the next head's units may not (their
                            # weights may still be in flight and a waiting
                            # matmul would head-of-line-block the PE queue)
                            if early_tasks:
                                early_tasks.pop(0)()
                            elif pending_tasks and (th, cp) != (0, 0):
                                pending_tasks.pop(0)()
                            extra = triggers.pop((h, th, cp, kt), None)
                            if extra:
                                pending_tasks.extend(extra)

                        def spill(po=po, po_sb=po_sb):
                            # po -> SBUF (DVE; the scalar engine stays
                            # dedicated to the exp stream)
                            for ci in range(2):
                                nc.vector.tensor_copy(po_sb[ci][0:HD+1, :],
                                                      po[ci][0:HD+1, :])

                        def norm(th=th, cp=cp, po_sb=po_sb, tok=tok, h=h):
                            # softmax normalization: o += po[c][:96] * (1/r_c)
                            oacc = oaccs[th]
                            psR = ps_big.tile([128, 1024], F32, tag="psb",
                                              name=f"psR_{h}_{th}_{cp}")
                            for ci in range(2):
                                nc.tensor.matmul(
                                    psR[0:HD, ci*512:(ci+1)*512],
                                    lhsT=sel[96:97, :],
                                    rhs=po_sb[ci][HD:HD+1, :],
                                    start=True, stop=True,
                                    tile_position=(96, 0))
                            rbc = spool.tile([128, 1024], F32, tag="rbc",
                                             name=f"rbc_{h}_{th}_{cp}")
                            nc.vector.reciprocal_approx_fast(
                                rbc[0:HD, :], psR[0:HD, :])
                            for ci in range(2):
                                idx = 2*cp + ci
                                if idx == 0:
                                    nc.vector.tensor_mul(
                                        oacc[0:HD, :],
                                        po_sb[ci][0:HD, :],
                                        rbc[0:HD, ci*512:(ci+1)*512])
                                else:
                                    tmp = spool.tile(
                                        [128, 512], BF16, tag="otmp",
                                        name=f"otmp_{h}_{th}_{cp}_{ci}")
                                    nc.vector.tensor_mul(
                                        tmp[0:HD, :], po_sb[ci][0:HD, :],
                                        rbc[0:HD, ci*512:(ci+1)*512])
                                    dst = (o_sb[0:HD, h, tok] if idx == 3
                                           else oacc[0:HD, :])
                                    nc.vector.tensor_add(
                                        dst, oacc[0:HD, :], tmp[0:HD, :])

                        pending_spill[0] = spill
                        pending_norm[0] = norm
                # last head: drain the AV/spill/norm pipeline before proj-C
                if h == HPC - 1:
                    while pending_av:
                        pending_av.pop(0)()
                    if pending_spill[0] is not None:
                        pending_spill[0]()
                        pending_spill[0] = None
                    if pending_norm[0] is not None:
                        pending_norm[0]()
                        pending_norm[0] = None

            def proj_c(tt):
                y_sb = y_pool.tile([128, DIM], BF16, tag="ysb",
                                   name=f"ysb_{tt}")
                for gh in range(2):
                    psY = ps_proj.tile([128, 512], F32, tag="psp",
                                       name=f"psY_{tt}_{gh}")
                    for hh in range(HPC):
                        nc.tensor.matmul(
                            psY[:, 0:384],
                            lhsT=o_sb[:, hh, tt*128:(tt+1)*128],
                            rhs=wp_sb[:, hh, gh*384:(gh+1)*384],
                            start=(hh == 0), stop=(hh == HPC - 1))
                    nc.vector.tensor_copy(y_sb[:, gh*384:(gh+1)*384],
                                          psY[:, 0:384])
                nc.sync.dma_start(y_d[tt*128:(tt+1)*128, :], y_sb[:])

            # proj-C for tokens 512-1023 is gated on the very last norm, so
            # the heads-0..2 part of its contraction runs early (during head
            # 3's attention) and only the head-3 matmul remains in the tail
            y012_sb = cpool.tile([128, 4, DIM], BF16)

            def proj_c_early(tt):
                for gh in range(2):
                    psY = ps_proj.tile([128, 512], F32, tag="psp",
                                       name=f"psY0_{tt}_{gh}")
                    for hh in range(HPC - 1):
                        nc.tensor.matmul(
                            psY[:, 0:384],
                            lhsT=o_sb[:, hh, tt*128:(tt+1)*128],
                            rhs=wp_sb[:, hh, gh*384:(gh+1)*384],
                            start=(hh == 0), stop=(hh == HPC - 2))
                    nc.vector.tensor_copy(y012_sb[:, tt-4, gh*384:(gh+1)*384],
                                          psY[:, 0:384])

            def proj_c_rest(tt):
                y_sb = y_pool.tile([128, DIM], BF16, tag="ysb",
                                   name=f"ysb_{tt}")
                for gh in range(2):
                    psY = ps_proj.tile([128, 512], F32, tag="psp",
                                       name=f"psY3_{tt}_{gh}")
                    nc.tensor.matmul(
                        psY[:, 0:384],
                        lhsT=o_sb[:, HPC-1, tt*128:(tt+1)*128],
                        rhs=wp_sb[:, HPC-1, gh*384:(gh+1)*384],
                        start=True, stop=True)
                    nc.vector.scalar_tensor_tensor(
                        y_sb[:, gh*384:(gh+1)*384],
                        psY[:, 0:384], 1.0,
                        y012_sb[:, tt-4, gh*384:(gh+1)*384],
                        op0=mybir.AluOpType.mult, op1=mybir.AluOpType.add)
                nc.sync.dma_start(y_d[tt*128:(tt+1)*128, :], y_sb[:])

            # tokens 0-511 (tt 0-3) are fully normalized once head 3's th=0
            # norms fire (early in its th=1 loop) -> run those proj-C tiles
            # inside head 3's attention instead of in the tail; the heads-0..2
            # partials for tt 4-7 (ready once head 2's th=1 norm fired, i.e.
            # during head 3's first block) run there too
            triggers[(HPC-1, 0, 1, 2)] = [lambda tt=tt: proj_c_early(tt)
                                          for tt in range(4, 8)]
            triggers[(HPC-1, 1, 0, 3)] = [lambda tt=tt: proj_c(tt)
                                          for tt in range(4)]

            for h in range(HPC):
                if h + 1 < HPC:
                    wa_sb = stream_weights(h + 1)
                    pending_tasks.extend(proj_tasks(h + 1, wa_sb))
                attention(h)
                # drain any leftover tasks before next head's attention
                while pending_tasks:
                    pending_tasks.pop(0)()

            # --- proj-C: remaining token tiles (head-3 part only) ---------------
            for tt in range(4, 8):
                proj_c_rest(tt)

    nc.compile()
    return nc


def _get_program():
    if "nc" not in _PROGRAM_CACHE:
        _PROGRAM_CACHE["nc"] = _build_program()
    return _PROGRAM_CACHE["nc"]


# ----------------------------------------------------------------------------
# NTFF profiling hook (axon containers without antenv.axon_hooks)
# ----------------------------------------------------------------------------

def _install_ntff_hook():
    """Provide antenv.axon_hooks backed by libaxon_pjrt.so so that
    run_bass_kernel_spmd(trace=True) can capture NTFF profiles under axon.
    Returns True if tracing is possible."""
    try:
        from antenv.axon_hooks import get_axon_ntff_profile_hook  # noqa: F401
        return True
    except ImportError:
        pass
    so_path = "/opt/axon/libaxon_pjrt.so"
    if not os.path.exists(so_path):
        return False
    lib = ctypes.CDLL(so_path)
    if not hasattr(lib, "axon_start_nrt_profile"):
        return False
    lib.axon_start_nrt_profile.argtypes = [
        ctypes.POINTER(ctypes.c_int64), ctypes.c_size_t]
    lib.axon_start_nrt_profile.restype = ctypes.c_int64
    lib.axon_stop_nrt_profile.argtypes = [ctypes.c_char_p]
    lib.axon_stop_nrt_profile.restype = ctypes.c_int64

    @contextlib.contextmanager
    def _hook(output_dir, device_ids):
        import jax
        jax.devices()
        if device_ids:
            ids = (ctypes.c_int64 * len(device_ids))(*device_ids)
            rc = lib.axon_start_nrt_profile(ids, len(device_ids))
        else:
            rc = lib.axon_start_nrt_profile(None, 0)
        if rc != 0:
            raise RuntimeError(f"axon_start_nrt_profile rc={rc}")
        try:
            yield
        finally:
            n = lib.axon_stop_nrt_profile(str(output_dir).encode())
            print(f"profile: {n} file(s) written to {output_dir}",
                  file=sys.stderr)

    mod = types.ModuleType("antenv.axon_hooks")
    _state = {"hook": _hook}
    mod.set_axon_ntff_profile_hook = lambda h: _state.__setitem__("hook", h)
    mod.get_axon_ntff_profile_hook = lambda: _state["hook"]
    sys.modules["antenv.axon_hooks"] = mod
    import antenv
    antenv.axon_hooks = mod
    return True


# ----------------------------------------------------------------------------
# Entry point
# ----------------------------------------------------------------------------

def kernel(trace=False, **inputs):
    nc = _get_program()
    in_maps, bp = _host_prepare(inputs)
    if trace:
        trace = _install_ntff_hook()
    res = bass_utils.run_bass_kernel_spmd(
        nc, in_maps, core_ids=list(range(NCORES)), trace=trace)
    y = np.empty((B, N, DIM), np.float32)
    for b in range(B):
        y[b] = (res.results[2*b]["y"].astype(np.float32)
                + res.results[2*b+1]["y"].astype(np.float32) + bp)
    if trace:
        kernel.last_results = res
    return y
